# revision 7
# baseline (speedup 1.0000x reference)
"""Causal self-attention Trainium2 kernel (8 NeuronCores, SPMD).

Sharding: 8 cores = 2 batches x 4 head-groups (4 heads of 64 dims each).
Each core computes full-sequence attention for its 4 heads plus the
partial output projection for its 256 y-columns; the host sums the 4
partials per batch and adds the output bias.

Layout strategy (no on-device transposes anywhere):
  - host supplies x[b].T as xT [C, T] (bf16)
  - qT, kT produced in [m, t] layout (W stationary, xT moving)
  - v produced in natural [t, m] layout (xT stationary, Wv moving),
    augmented with a ones column per head (M=65) so the attention-value
    matmul also emits the softmax denominator row for free
  - attT[j, i] = sum_d kT[d,j] qT[d,i]  (kT stationary K=64; two heads
    run concurrently via row-tiled base partitions 0/64)
  - exp on ScalarE (fused 1/sqrt(64) scale) in 1024-wide chunks;
    diagonal 128-tiles masked with a multiplicative mask tile
  - PT rows stored triangularly (only i >= 128*jt) — fits in SBUF
    alongside xT, letting the second head-pair's q/k projections
    interleave into pair 0's scalar-bound attention stretch
  - yT_aug[65, i] accumulated over j-tiles; row 64 = softmax sum
  - 1/s via DVE reciprocal_approx_fast on a partition-0 row, broadcast
    with a K=1 ones matmul; pair-0 rows staged early so pair-1
    normalization has no gather latency
  - pair-1 pipeline: S-broadcast+scale for i-block N run while the
    attention-value pass for N+1 runs on the PE; out-proj follows
  - PE warm-up: dummy matmuls during the initial DMA wait keep the HAM
    clock gate open so real work starts at 2.4 GHz
  - out[t, n] partials stored bf16; host sums the 4 partials in f32
"""

import sys

for _p in ("/opt/trn_rl_repo",):
    if _p not in sys.path:
        sys.path.insert(0, _p)

from contextlib import ExitStack

import ml_dtypes
import numpy as np

import concourse.bass as bass
import concourse.tile as tile
from concourse import bacc, mybir
from concourse.bass_utils import run_bass_kernel_spmd

BF16 = mybir.dt.bfloat16
F32 = mybir.dt.float32
NP_BF16 = ml_dtypes.bfloat16

B, T, C = 2, 2048, 1024
H, D = 16, 64
N_CORES = 8
GROUPS = 4          # head groups (cores per batch)
MH = C // GROUPS    # 256 columns per core (4 heads)
LH = MH // D        # 4 local heads
CT = C // 128       # 8 contraction tiles
TT = T // 128       # 16 sequence tiles of 128
IB = T // 512       # 4 i-blocks of 512
SCALE = 1.0 / np.sqrt(D)
N_WARM = 22         # dummy matmuls covering the initial DMA wait

# triangular PT layout: row jt holds cols i in [128*jt, T)
TRI_OFF = [0] * TT
for _jt in range(1, TT):
    TRI_OFF[_jt] = TRI_OFF[_jt - 1] + (T - 128 * (_jt - 1))
TRI = TRI_OFF[-1] + (T - 128 * (TT - 1))   # 17408


def _causal_mask() -> np.ndarray:
    """mask[j, i] = 1.0 if j <= i else 0 (bf16), [128, 128]."""
    j = np.arange(128)[:, None]
    i = np.arange(128)[None, :]
    return (j <= i).astype(NP_BF16)


def emit_kernel(nc, xT_d, wq_d, wk_d, wv_d, wp_d, bq_d, bk_d, bv_d, out_d, mask_d):
    with tile.TileContext(nc) as tc, ExitStack() as ctx:
        # ---- long-lived tiles -------------------------------------------
        keep = ctx.enter_context(tc.tile_pool(name="keep", bufs=1))
        qT_s = keep.tile([128, 2, T], BF16, tag="qT")
        kT_s = keep.tile([128, 2, T], BF16, tag="kT")
        v_s = keep.tile([128, TT, LH, D + 1], BF16, tag="v")
        yTn_s = keep.tile([128, 2, T], BF16, tag="yTn")
        wp_s = keep.tile([128, 2, C], BF16, tag="wp")
        mask_st = keep.tile([128, 128], BF16, tag="mask_st")
        mask_s = keep.tile([128, 128], BF16, tag="mask")
        bq_st = keep.tile([128, 2], F32, tag="bq_st")
        bq_s = keep.tile([128, 2], F32, tag="bq")
        bk_st = keep.tile([128, 2], F32, tag="bk_st")
        bk_s = keep.tile([128, 2], F32, tag="bk")
        bv_row = keep.tile([1, MH], F32, tag="bv_row")
        bv_row_bf = keep.tile([1, MH], BF16, tag="bv_row_bf")
        bv_bc = keep.tile([128, MH], F32, tag="bv_bc")
        ones_bf128 = keep.tile([1, 128], BF16, tag="ones_bf128")
        ones_bf = keep.tile([1, 64], BF16, tag="ones_bf")
        rs_all = keep.tile([1, 2, IB, 2, 512], BF16, tag="rs_all")  # 1/s rows
        warm_s = keep.tile([128, 256], BF16, tag="warm")

        nc.vector.memset(ones_bf128[:], 1.0)
        nc.vector.memset(ones_bf[:], 1.0)
        nc.vector.memset(warm_s[:], 1.0)
        nc.vector.memset(v_s[:, :, :, D : D + 1], 1.0)

        # projection inputs stay alive through pair 0's attention so the
        # mt=1 q/k projections can interleave there
        pin = ctx.enter_context(tc.tile_pool(name="proj_in", bufs=1))
        xT_s = pin.tile([128, CT, T], BF16, tag="xT")
        wq_s = pin.tile([128, CT, MH], BF16, tag="wq")
        wk_s = pin.tile([128, CT, MH], BF16, tag="wk")
        wv_s = pin.tile([128, CT, MH], BF16, tag="wv")
        xT_r = xT_d.ap().rearrange("(o p) t -> p o t", p=128)
        wq_r = wq_d.ap().rearrange("(o p) m -> p o m", p=128)
        wk_r = wk_d.ap().rearrange("(o p) m -> p o m", p=128)
        wv_r = wv_d.ap().rearrange("(o p) m -> p o m", p=128)

        # weights on the scalar DMA queue, xT on sync: the two streams
        # land concurrently so the first matmul group starts sooner
        def xt_chunk(tb):
            nc.sync.dma_start(
                xT_s[:, :, tb * 512 : (tb + 1) * 512],
                xT_r[:, :, tb * 512 : (tb + 1) * 512],
            )

        nc.scalar.dma_start(wq_s[:], wq_r[:])
        xt_chunk(0)
        nc.scalar.dma_start(wk_s[:], wk_r[:])
        xt_chunk(1)
        nc.scalar.dma_start(wv_s[:], wv_r[:])
        xt_chunk(2)
        xt_chunk(3)
        wp_r = wp_d.ap().rearrange("(o p) n -> p o n", p=128)
        nc.scalar.dma_start(wp_s[:], wp_r[:])
        # consts staged through a DVE copy: consumers then depend on DVE
        # program order instead of a DMA semaphore (walrus 1-wait limit)
        nc.gpsimd.dma_start(mask_st[:], mask_d.ap())
        nc.gpsimd.dma_start(bq_st[:], bq_d.ap().rearrange("(o p) -> p o", p=128))
        nc.gpsimd.dma_start(bk_st[:], bk_d.ap().rearrange("(o p) -> p o", p=128))
        nc.gpsimd.dma_start(bv_row[:], bv_d.ap()[None, :])
        nc.vector.tensor_copy(mask_s[:], mask_st[:])
        nc.vector.tensor_copy(bq_s[:], bq_st[:])
        nc.vector.tensor_copy(bk_s[:], bk_st[:])
        nc.vector.tensor_copy(bv_row_bf[:], bv_row[:])

        def proj_group(ps, w_s, b_s, dst, mt, tb):
            """one [128, 512] column block of qT or kT (8-deep K accum)."""
            for ct in range(CT):
                nc.tensor.matmul(
                    ps[:, 0:512],
                    w_s[:, ct, mt * 128 : (mt + 1) * 128],
                    xT_s[:, ct, tb * 512 : (tb + 1) * 512],
                    start=(ct == 0),
                    stop=(ct == CT - 1),
                )
            nc.vector.tensor_scalar(
                dst[:, mt, tb * 512 : (tb + 1) * 512],
                ps[:, 0:512],
                b_s[:, mt : mt + 1],
                None,
                mybir.AluOpType.add,
            )

        # ---- phase 1: mt=0 projections + v ------------------------------
        with tc.tile_pool(name="proj_ps", bufs=4, space="PSUM") as pps:
            # PE warm-up: dummy matmuls cover the DMA wait so the HAM clock
            # gate opens (K=8/8) before real data lands
            for _ in range(N_WARM):
                wps_t = pps.tile([128, 512], F32, tag="proj_ps", name="warm_ps")
                nc.tensor.matmul(
                    wps_t[:, 0:256], warm_s[:, 0:128], warm_s[:], start=True, stop=True
                )

            bv_ps = pps.tile([128, MH], F32, tag="v_ps", name="bv_ps")
            nc.tensor.matmul(
                bv_ps[:], ones_bf128[:], bv_row_bf[:], start=True, stop=True
            )
            nc.vector.tensor_copy(bv_bc[:], bv_ps[:])

            for w_s, b_s, dst in ((wq_s, bq_s, qT_s), (wk_s, bk_s, kT_s)):
                for tb in range(IB):
                    ps = pps.tile([128, 512], F32, tag="proj_ps")
                    proj_group(ps, w_s, b_s, dst, 0, tb)

            # v natural [t, m]  (xT stationary)
            for tt in range(TT):
                ps = pps.tile([128, MH], F32, tag="v_ps")
                for ct in range(CT):
                    nc.tensor.matmul(
                        ps[:],
                        xT_s[:, ct, tt * 128 : (tt + 1) * 128],
                        wv_s[:, ct, :],
                        start=(ct == 0),
                        stop=(ct == CT - 1),
                    )
                nc.vector.tensor_tensor(
                    v_s[:, tt, :, 0:D],
                    ps[:].rearrange("p (h d) -> p h d", h=LH),
                    bv_bc[:].rearrange("p (h d) -> p h d", h=LH),
                    mybir.AluOpType.add,
                )

        # ---- phase 2+3: attention, mt=1 proj + out-proj interleaved ------
        with (
            tc.tile_pool(name="pt", bufs=1) as ptp,
            tc.tile_pool(name="att_ps", bufs=2, space="PSUM") as aps,
            tc.tile_pool(name="yt_ps", bufs=2, space="PSUM") as yps,
            tc.tile_pool(name="out_ps", bufs=2, space="PSUM") as ops,
            tc.tile_pool(name="norm", bufs=2) as npool,
            tc.tile_pool(name="out_sb", bufs=3) as osb,
        ):
            out_r = out_d.ap().rearrange("(tt p) n -> tt p n", p=128)
            yTu_by_p = [None, None]
            # mt=1 q/k projection groups, interleaved into pair 0's loop
            mt1_groups = [(wq_s, bq_s, qT_s, tb) for tb in range(IB)] + [
                (wk_s, bk_s, kT_s, tb) for tb in range(IB)
            ]

            for p in range(2):
                PT = [
                    ptp.tile([128, TRI], BF16, tag=f"PT{lh}", name=f"PT{lh}")
                    for lh in range(2)
                ]
                yTu = npool.tile([64, 8, 512], BF16, tag="yTu", name="yTu")
                yTu_by_p[p] = yTu

                def attT_row(jt):
                    """attT + exp (+ diagonal mask) for j-tile jt, i >= 128*jt."""
                    ia = 128 * jt
                    base = TRI_OFF[jt]
                    w_all = T - ia
                    off = 0
                    while off < w_all:
                        cw = min(1024, w_all - off)
                        for lh in range(2):
                            att_ps = aps.tile([128, 1024], F32, tag="att_ps")
                            prow = slice(64 * lh, 64 * lh + 64)
                            for s5 in range(0, cw, 512):
                                nn = min(512, cw - s5)
                                nc.tensor.matmul(
                                    att_ps[:, s5 : s5 + nn],
                                    kT_s[prow, p, jt * 128 : (jt + 1) * 128],
                                    qT_s[prow, p, ia + off + s5 : ia + off + s5 + nn],
                                    start=True,
                                    stop=True,
                                )
                            nc.scalar.activation(
                                PT[lh][:, base + off : base + off + cw],
                                att_ps[:, :cw],
                                mybir.ActivationFunctionType.Exp,
                                scale=float(SCALE),
                            )
                            if off == 0:
                                # diagonal 128x128 tile: zero j > i
                                nc.vector.tensor_tensor(
                                    PT[lh][:, base : base + 128],
                                    PT[lh][:, base : base + 128],
                                    mask_s[:],
                                    mybir.AluOpType.mult,
                                )
                        off += cw

                def av_block(ib):
                    """attention @ v for i-block ib; returns yT_ps pair."""
                    yT_ps = [
                        yps.tile([D + 1, 512], F32, tag="yT_ps", name=f"yT_ps{lh}")
                        for lh in range(2)
                    ]
                    for jt in range(4 * ib + 4):
                        for lh in range(2):
                            ia = 128 * jt
                            c0 = max(512 * ib, ia)
                            nc.tensor.matmul(
                                yT_ps[lh][:, c0 - 512 * ib : 512],
                                v_s[:, jt, 2 * p + lh, :],
                                PT[lh][
                                    :,
                                    TRI_OFF[jt]
                                    + c0
                                    - ia : TRI_OFF[jt]
                                    + 512 * ib
                                    + 512
                                    - ia,
                                ],
                                start=(jt == 0),
                                stop=(jt == 4 * ib + 3),
                            )
                    return yT_ps

                def stash_recip(ib, yT_ps):
                    """stash y (bf16) + denominators; 1/s onto partition 0."""
                    for lh in range(2):
                        nc.vector.tensor_copy(yTu[:, ib * 2 + lh, :], yT_ps[lh][0:D, :])
                    st = npool.tile([1, 1024], F32, tag="st", name="st", bufs=1)
                    for lh in range(2):
                        nc.vector.tensor_copy(
                            st[0:1, lh * 512 : (lh + 1) * 512], yT_ps[lh][D : D + 1, :]
                        )
                    rf = npool.tile([1, 1024], F32, tag="rf", name="rf", bufs=1)
                    nc.vector.reciprocal_approx_fast(rf[:], st[:])
                    with nc.allow_low_precision(
                        reason="1/s broadcast via bf16 matmul; ~0.4% noise ok"
                    ):
                        nc.vector.tensor_copy(
                            rs_all[0:1, p, ib, :, :].rearrange("a l c -> a (l c)"),
                            rf[:],
                        )

                def s_mults(ib):
                    """broadcast 1/s (K=1 ones matmul) and scale into yTn."""
                    for r in range(4):
                        pp, lh = divmod(r, 2)
                        S_ps = yps.tile([D + 1, 512], F32, tag="yT_ps", name="S_ps")
                        nc.tensor.matmul(
                            S_ps[0:64, :],
                            ones_bf[:],
                            rs_all[0:1, pp, ib, lh, :],
                            start=True,
                            stop=True,
                        )
                        nc.vector.tensor_tensor(
                            yTn_s[64 * lh : 64 * lh + 64, pp, 512 * ib : 512 * ib + 512],
                            yTu_by_p[pp][:, ib * 2 + lh, :],
                            S_ps[0:64, :],
                            mybir.AluOpType.mult,
                        )

                def outproj(ib):
                    for tt in range(4 * ib, 4 * ib + 4):
                        for nb in range(2):
                            o_ps = ops.tile([128, 512], F32, tag="out_ps", name="o_ps")
                            for pp in range(2):
                                nc.tensor.matmul(
                                    o_ps[:],
                                    yTn_s[:, pp, tt * 128 : (tt + 1) * 128],
                                    wp_s[:, pp, nb * 512 : (nb + 1) * 512],
                                    start=(pp == 0),
                                    stop=(pp == 1),
                                )
                            ot = osb.tile([128, 512], BF16, tag="out_t")
                            with nc.allow_low_precision(
                                reason="bf16 output partials; host sums in f32"
                            ):
                                nc.vector.tensor_copy(ot[:], o_ps[:])
                            nc.sync.dma_start(
                                out_r[tt, :, nb * 512 : (nb + 1) * 512], ot[:]
                            )

                if p == 0:
                    for ib in range(IB):
                        for jt in range(4 * ib, 4 * ib + 4):
                            attT_row(jt)
                        yT_ps = av_block(ib)
                        stash_recip(ib, yT_ps)
                        # second pair's q/k projections fill the PE while
                        # ScalarE works through the exp backlog
                        for _ in range(2):
                            if mt1_groups:
                                w_s, b_s, dst, tb = mt1_groups.pop(0)
                                ps = aps.tile([128, 1024], F32, tag="att_ps")
                                proj_group(ps, w_s, b_s, dst, 1, tb)
                else:
                    # prologue: two i-blocks of attT ahead, first av staged
                    for jt in range(0, 4):
                        attT_row(jt)
                    yT_ps = av_block(0)
                    stash_recip(0, yT_ps)
                    for jt in range(4, 8):
                        attT_row(jt)
                    for ib in range(IB):
                        s_mults(ib)
                        if ib + 1 < IB:
                            yT_ps = av_block(ib + 1)
                            stash_recip(ib + 1, yT_ps)
                        if ib + 2 < IB:
                            for jt in range(4 * (ib + 2), 4 * (ib + 2) + 4):
                                attT_row(jt)
                        outproj(ib)


_NC_CACHE = None


def get_nc() -> bass.Bass:
    global _NC_CACHE
    if _NC_CACHE is None:
        nc = bacc.Bacc()
        xT_d = nc.declare_dram_parameter("xT", [C, T], BF16, isOutput=False)
        wq_d = nc.declare_dram_parameter("wq", [C, MH], BF16, isOutput=False)
        wk_d = nc.declare_dram_parameter("wk", [C, MH], BF16, isOutput=False)
        wv_d = nc.declare_dram_parameter("wv", [C, MH], BF16, isOutput=False)
        wp_d = nc.declare_dram_parameter("wp", [MH, C], BF16, isOutput=False)
        bq_d = nc.declare_dram_parameter("bq", [MH], F32, isOutput=False)
        bk_d = nc.declare_dram_parameter("bk", [MH], F32, isOutput=False)
        bv_d = nc.declare_dram_parameter("bv", [MH], F32, isOutput=False)
        out_d = nc.declare_dram_parameter("out", [T, C], BF16, isOutput=True)
        mask_d = nc.inline_tensor(_causal_mask(), name="causal_mask")
        emit_kernel(
            nc, xT_d, wq_d, wk_d, wv_d, wp_d, bq_d, bk_d, bv_d, out_d, mask_d
        )
        nc.finalize()
        _NC_CACHE = nc
    return _NC_CACHE


def make_in_maps(x, Wq, bq, Wk, bk, Wv, bv, Wp, bp):
    in_maps = []
    for core in range(N_CORES):
        b, g = divmod(core, GROUPS)
        sl = slice(g * MH, (g + 1) * MH)
        in_maps.append(
            {
                "xT": np.ascontiguousarray(x[b].T).astype(NP_BF16),
                "wq": np.ascontiguousarray(Wq[:, sl]).astype(NP_BF16),
                "wk": np.ascontiguousarray(Wk[:, sl]).astype(NP_BF16),
                "wv": np.ascontiguousarray(Wv[:, sl]).astype(NP_BF16),
                "wp": np.ascontiguousarray(Wp[sl, :]).astype(NP_BF16),
                "bq": np.ascontiguousarray(bq[sl]).astype(np.float32),
                "bk": np.ascontiguousarray(bk[sl]).astype(np.float32),
                "bv": np.ascontiguousarray(bv[sl]).astype(np.float32),
            }
        )
    return in_maps


def kernel(x, Wq, bq, Wk, bk, Wv, bv, Wp, bp, _results_hook=None, _trace=False):
    x = np.asarray(x, dtype=np.float32)
    nc = get_nc()
    in_maps = make_in_maps(x, Wq, bq, Wk, bk, Wv, bv, Wp, bp)
    res = run_bass_kernel_spmd(
        nc, in_maps, core_ids=list(range(N_CORES)), trace=_trace
    )
    if _results_hook is not None:
        _results_hook(res)
    out = np.zeros((B, T, C), dtype=np.float32)
    for core in range(N_CORES):
        b = core // GROUPS
        out[b] += np.asarray(res.results[core]["out"], dtype=np.float32)
    out += np.asarray(bp, dtype=np.float32)[None, None, :]
    return out


# revision 8
# speedup vs baseline: 1.2556x; 1.2556x over previous
"""Causal self-attention Trainium2 kernel (8 NeuronCores, SPMD).

Sharding: 8 cores = 2 batches x 4 head-groups (4 heads of 64 dims each).
Each core computes full-sequence attention for its 4 heads plus the
partial output projection for its 256 y-columns; the host sums the 4
partials per batch and adds the output bias.

Layout strategy (no on-device transposes anywhere):
  - host supplies x[b].T as xT [C, T] (bf16)
  - qT, kT produced in [m, t] layout (W stationary, xT moving)
  - v produced in natural [t, m] layout (xT stationary, Wv moving),
    augmented with a ones column per head (M=65) so the attention-value
    matmul also emits the softmax denominator row for free
  - attT[j, i] = sum_d kT[d,j] qT[d,i]  (kT stationary K=64; two heads
    run concurrently via row-tiled base partitions 0/64)
  - exp on ScalarE (fused 1/sqrt(64) scale) in 1024-wide chunks;
    diagonal 128-tiles masked with a multiplicative mask tile
  - PT rows stored triangularly (only i >= 128*jt) — fits in SBUF
    alongside xT, letting the second head-pair's q/k projections
    interleave into pair 0's scalar-bound attention stretch
  - yT_aug[65, i] accumulated over j-tiles; row 64 = softmax sum
  - 1/s via DVE reciprocal_approx_fast on a partition-0 row, broadcast
    with a K=1 ones matmul; pair-0 rows staged early so pair-1
    normalization has no gather latency
  - pair-1 pipeline: S-broadcast+scale for i-block N run while the
    attention-value pass for N+1 runs on the PE; out-proj follows
  - PE warm-up: dummy matmuls during the initial DMA wait keep the HAM
    clock gate open so real work starts at 2.4 GHz
  - out[t, n] partials stored bf16; host sums the 4 partials in f32
"""

import sys

for _p in ("/opt/trn_rl_repo",):
    if _p not in sys.path:
        sys.path.insert(0, _p)

from contextlib import ExitStack

import ml_dtypes
import numpy as np

import concourse.bass as bass
import concourse.tile as tile
from concourse import bacc, mybir
from concourse.bass_utils import run_bass_kernel_spmd

BF16 = mybir.dt.bfloat16
F32 = mybir.dt.float32
NP_BF16 = ml_dtypes.bfloat16

B, T, C = 2, 2048, 1024
H, D = 16, 64
N_CORES = 8
GROUPS = 4          # head groups (cores per batch)
MH = C // GROUPS    # 256 columns per core (4 heads)
LH = MH // D        # 4 local heads
CT = C // 128       # 8 contraction tiles
TT = T // 128       # 16 sequence tiles of 128
IB = T // 512       # 4 i-blocks of 512
SCALE = 1.0 / np.sqrt(D)
N_WARM = 22         # dummy matmuls covering the initial DMA wait

# triangular PT layout: row jt holds cols i in [128*jt, T)
TRI_OFF = [0] * TT
for _jt in range(1, TT):
    TRI_OFF[_jt] = TRI_OFF[_jt - 1] + (T - 128 * (_jt - 1))
TRI = TRI_OFF[-1] + (T - 128 * (TT - 1))   # 17408


def _causal_mask() -> np.ndarray:
    """mask[j, i] = 1.0 if j <= i else 0 (bf16), [128, 128]."""
    j = np.arange(128)[:, None]
    i = np.arange(128)[None, :]
    return (j <= i).astype(NP_BF16)


def emit_kernel(nc, xT_d, wq_d, wk_d, wv_d, wp_d, bq_d, bk_d, bv_d, out_d, mask_d):
    with tile.TileContext(nc) as tc, ExitStack() as ctx:
        # ---- long-lived tiles -------------------------------------------
        keep = ctx.enter_context(tc.tile_pool(name="keep", bufs=1))
        qT_s = keep.tile([128, 2, T], BF16, tag="qT")
        kT_s = keep.tile([128, 2, T], BF16, tag="kT")
        v_s = keep.tile([128, TT, LH, D + 1], BF16, tag="v")
        yTn_s = keep.tile([128, 2, T], BF16, tag="yTn")
        wp_s = keep.tile([128, 2, C], BF16, tag="wp")
        mask_st = keep.tile([128, 128], BF16, tag="mask_st")
        mask_s = keep.tile([128, 128], BF16, tag="mask")
        bq_st = keep.tile([128, 2], F32, tag="bq_st")
        bq_s = keep.tile([128, 2], F32, tag="bq")
        bk_st = keep.tile([128, 2], F32, tag="bk_st")
        bk_s = keep.tile([128, 2], F32, tag="bk")
        bv_row = keep.tile([1, MH], F32, tag="bv_row")
        bv_row_bf = keep.tile([1, MH], BF16, tag="bv_row_bf")
        bv_bc = keep.tile([128, MH], F32, tag="bv_bc")
        ones_bf128 = keep.tile([1, 128], BF16, tag="ones_bf128")
        ones_bf = keep.tile([1, 64], BF16, tag="ones_bf")
        rs_all = keep.tile([1, 2, IB, 2, 512], BF16, tag="rs_all")  # 1/s rows
        warm_s = keep.tile([128, 256], BF16, tag="warm")

        nc.vector.memset(ones_bf128[:], 1.0)
        nc.vector.memset(ones_bf[:], 1.0)
        nc.vector.memset(warm_s[:], 1.0)
        nc.vector.memset(v_s[:, :, :, D : D + 1], 1.0)

        # projection inputs stay alive through pair 0's attention so the
        # mt=1 q/k projections can interleave there
        pin = ctx.enter_context(tc.tile_pool(name="proj_in", bufs=1))
        xT_s = pin.tile([128, CT, T], BF16, tag="xT")
        wq_s = pin.tile([128, CT, MH], BF16, tag="wq")
        wk_s = pin.tile([128, CT, MH], BF16, tag="wk")
        wv_s = pin.tile([128, CT, MH], BF16, tag="wv")
        xT_r = xT_d.ap().rearrange("(o p) t -> p o t", p=128)
        wq_r = wq_d.ap().rearrange("(o p) m -> p o m", p=128)
        wk_r = wk_d.ap().rearrange("(o p) m -> p o m", p=128)
        wv_r = wv_d.ap().rearrange("(o p) m -> p o m", p=128)

        # weights on the scalar DMA queue, xT on sync: the two streams
        # land concurrently so the first matmul group starts sooner
        def xt_chunk(tb):
            nc.sync.dma_start(
                xT_s[:, :, tb * 512 : (tb + 1) * 512],
                xT_r[:, :, tb * 512 : (tb + 1) * 512],
            )

        nc.scalar.dma_start(wq_s[:], wq_r[:])
        xt_chunk(0)
        nc.scalar.dma_start(wk_s[:], wk_r[:])
        xt_chunk(1)
        nc.scalar.dma_start(wv_s[:], wv_r[:])
        xt_chunk(2)
        xt_chunk(3)
        wp_r = wp_d.ap().rearrange("(o p) n -> p o n", p=128)
        nc.scalar.dma_start(wp_s[:], wp_r[:])
        # consts staged through a DVE copy: consumers then depend on DVE
        # program order instead of a DMA semaphore (walrus 1-wait limit)
        nc.gpsimd.dma_start(mask_st[:], mask_d.ap())
        nc.gpsimd.dma_start(bq_st[:], bq_d.ap().rearrange("(o p) -> p o", p=128))
        nc.gpsimd.dma_start(bk_st[:], bk_d.ap().rearrange("(o p) -> p o", p=128))
        nc.gpsimd.dma_start(bv_row[:], bv_d.ap()[None, :])
        nc.vector.tensor_copy(mask_s[:], mask_st[:])
        nc.vector.tensor_copy(bq_s[:], bq_st[:])
        nc.vector.tensor_copy(bk_s[:], bk_st[:])
        nc.vector.tensor_copy(bv_row_bf[:], bv_row[:])

        def proj_group(ps, w_s, b_s, dst, mt, tb):
            """one [128, 512] column block of qT or kT (8-deep K accum)."""
            for ct in range(CT):
                nc.tensor.matmul(
                    ps[:, 0:512],
                    w_s[:, ct, mt * 128 : (mt + 1) * 128],
                    xT_s[:, ct, tb * 512 : (tb + 1) * 512],
                    start=(ct == 0),
                    stop=(ct == CT - 1),
                )
            nc.vector.tensor_scalar(
                dst[:, mt, tb * 512 : (tb + 1) * 512],
                ps[:, 0:512],
                b_s[:, mt : mt + 1],
                None,
                mybir.AluOpType.add,
            )

        # ---- phase 1: mt=0 projections + v ------------------------------
        with tc.tile_pool(name="proj_ps", bufs=4, space="PSUM") as pps:
            # PE warm-up: dummy matmuls cover the DMA wait so the HAM clock
            # gate opens (K=8/8) before real data lands
            for _ in range(N_WARM):
                wps_t = pps.tile([128, 512], F32, tag="proj_ps", name="warm_ps")
                nc.tensor.matmul(
                    wps_t[:, 0:256], warm_s[:, 0:128], warm_s[:], start=True, stop=True
                )

            bv_ps = pps.tile([128, MH], F32, tag="v_ps", name="bv_ps")
            nc.tensor.matmul(
                bv_ps[:], ones_bf128[:], bv_row_bf[:], start=True, stop=True
            )
            nc.vector.tensor_copy(bv_bc[:], bv_ps[:])

            for w_s, b_s, dst in ((wq_s, bq_s, qT_s), (wk_s, bk_s, kT_s)):
                for tb in range(IB):
                    ps = pps.tile([128, 512], F32, tag="proj_ps")
                    proj_group(ps, w_s, b_s, dst, 0, tb)

            # v natural [t, m]  (xT stationary)
            for tt in range(TT):
                ps = pps.tile([128, MH], F32, tag="v_ps")
                for ct in range(CT):
                    nc.tensor.matmul(
                        ps[:],
                        xT_s[:, ct, tt * 128 : (tt + 1) * 128],
                        wv_s[:, ct, :],
                        start=(ct == 0),
                        stop=(ct == CT - 1),
                    )
                nc.vector.tensor_tensor(
                    v_s[:, tt, :, 0:D],
                    ps[:].rearrange("p (h d) -> p h d", h=LH),
                    bv_bc[:].rearrange("p (h d) -> p h d", h=LH),
                    mybir.AluOpType.add,
                )

        # ---- phase 2+3: attention, mt=1 proj + out-proj interleaved ------
        with (
            tc.tile_pool(name="pt", bufs=1) as ptp,
            tc.tile_pool(name="att_ps", bufs=2, space="PSUM") as aps,
            tc.tile_pool(name="yt_ps", bufs=2, space="PSUM") as yps,
            tc.tile_pool(name="out_ps", bufs=2, space="PSUM") as ops,
            tc.tile_pool(name="norm", bufs=2) as npool,
            tc.tile_pool(name="out_sb", bufs=3) as osb,
        ):
            out_r = out_d.ap().rearrange("(tt p) n -> tt p n", p=128)
            yTu_by_p = [None, None]
            # mt=1 q/k projection groups, interleaved into pair 0's loop
            mt1_groups = [(wq_s, bq_s, qT_s, tb) for tb in range(IB)] + [
                (wk_s, bk_s, kT_s, tb) for tb in range(IB)
            ]

            for p in range(2):
                PT = [
                    ptp.tile([128, TRI], BF16, tag=f"PT{lh}", name=f"PT{lh}")
                    for lh in range(2)
                ]
                yTu = npool.tile([64, 8, 512], BF16, tag="yTu", name="yTu")
                yTu_by_p[p] = yTu

                def attT_row(jt):
                    """attT + exp (+ diagonal mask) for j-tile jt, i >= 128*jt."""
                    ia = 128 * jt
                    base = TRI_OFF[jt]
                    w_all = T - ia
                    off = 0
                    while off < w_all:
                        cw = min(1024, w_all - off)
                        for lh in range(2):
                            att_ps = aps.tile([128, 1024], F32, tag="att_ps")
                            prow = slice(64 * lh, 64 * lh + 64)
                            for s5 in range(0, cw, 512):
                                nn = min(512, cw - s5)
                                nc.tensor.matmul(
                                    att_ps[:, s5 : s5 + nn],
                                    kT_s[prow, p, jt * 128 : (jt + 1) * 128],
                                    qT_s[prow, p, ia + off + s5 : ia + off + s5 + nn],
                                    start=True,
                                    stop=True,
                                )
                            nc.scalar.activation(
                                PT[lh][:, base + off : base + off + cw],
                                att_ps[:, :cw],
                                mybir.ActivationFunctionType.Exp,
                                scale=float(SCALE),
                            )
                            if off == 0:
                                # diagonal 128x128 tile: zero j > i
                                nc.vector.tensor_tensor(
                                    PT[lh][:, base : base + 128],
                                    PT[lh][:, base : base + 128],
                                    mask_s[:],
                                    mybir.AluOpType.mult,
                                )
                        off += cw

                def av_block(ib):
                    """attention @ v for i-block ib; returns yT_ps pair."""
                    yT_ps = [
                        yps.tile([D + 1, 512], F32, tag="yT_ps", name=f"yT_ps{lh}")
                        for lh in range(2)
                    ]
                    for jt in range(4 * ib + 4):
                        for lh in range(2):
                            ia = 128 * jt
                            c0 = max(512 * ib, ia)
                            nc.tensor.matmul(
                                yT_ps[lh][:, c0 - 512 * ib : 512],
                                v_s[:, jt, 2 * p + lh, :],
                                PT[lh][
                                    :,
                                    TRI_OFF[jt]
                                    + c0
                                    - ia : TRI_OFF[jt]
                                    + 512 * ib
                                    + 512
                                    - ia,
                                ],
                                start=(jt == 0),
                                stop=(jt == 4 * ib + 3),
                            )
                    return yT_ps

                def stash_recip(ib, yT_ps):
                    """stash y (bf16) + denominators; 1/s onto partition 0."""
                    for lh in range(2):
                        nc.vector.tensor_copy(yTu[:, ib * 2 + lh, :], yT_ps[lh][0:D, :])
                    st = npool.tile([1, 1024], F32, tag="st", name="st", bufs=1)
                    for lh in range(2):
                        nc.vector.tensor_copy(
                            st[0:1, lh * 512 : (lh + 1) * 512], yT_ps[lh][D : D + 1, :]
                        )
                    rf = npool.tile([1, 1024], F32, tag="rf", name="rf", bufs=1)
                    nc.vector.reciprocal_approx_fast(rf[:], st[:])
                    with nc.allow_low_precision(
                        reason="1/s broadcast via bf16 matmul; ~0.4% noise ok"
                    ):
                        nc.vector.tensor_copy(
                            rs_all[0:1, p, ib, :, :].rearrange("a l c -> a (l c)"),
                            rf[:],
                        )

                def s_mults(ib):
                    """broadcast 1/s (K=1 ones matmul) and scale into yTn."""
                    for r in range(4):
                        pp, lh = divmod(r, 2)
                        S_ps = ops.tile([128, 512], F32, tag="out_ps", name="S_ps")
                        nc.tensor.matmul(
                            S_ps[0:64, :],
                            ones_bf[:],
                            rs_all[0:1, pp, ib, lh, :],
                            start=True,
                            stop=True,
                        )
                        nc.vector.tensor_tensor(
                            yTn_s[64 * lh : 64 * lh + 64, pp, 512 * ib : 512 * ib + 512],
                            yTu_by_p[pp][:, ib * 2 + lh, :],
                            S_ps[0:64, :],
                            mybir.AluOpType.mult,
                        )

                def outproj(ib):
                    for tt in range(4 * ib, 4 * ib + 4):
                        for nb in range(2):
                            o_ps = ops.tile([128, 512], F32, tag="out_ps", name="o_ps")
                            for pp in range(2):
                                nc.tensor.matmul(
                                    o_ps[:],
                                    yTn_s[:, pp, tt * 128 : (tt + 1) * 128],
                                    wp_s[:, pp, nb * 512 : (nb + 1) * 512],
                                    start=(pp == 0),
                                    stop=(pp == 1),
                                )
                            ot = osb.tile([128, 512], BF16, tag="out_t")
                            with nc.allow_low_precision(
                                reason="bf16 output partials; host sums in f32"
                            ):
                                nc.vector.tensor_copy(ot[:], o_ps[:])
                            nc.sync.dma_start(
                                out_r[tt, :, nb * 512 : (nb + 1) * 512], ot[:]
                            )

                def mt1_fill(n):
                    # second pair's q/k projections fill the PE while
                    # ScalarE works through the exp backlog
                    for _ in range(n):
                        if mt1_groups:
                            w_s, b_s, dst, tb = mt1_groups.pop(0)
                            ps = aps.tile([128, 1024], F32, tag="att_ps")
                            proj_group(ps, w_s, b_s, dst, 1, tb)

                if p == 0:
                    for ib in range(IB):
                        for jt in range(4 * ib, 4 * ib + 4):
                            attT_row(jt)
                        mt1_fill(2 if ib < 2 else 1)
                        yT_ps = av_block(ib)
                        stash_recip(ib, yT_ps)
                else:
                    # prologue: two i-blocks of attT ahead, first av staged;
                    # the late kT mt=1 groups land here (needed only from
                    # attT row 8 onward)
                    for jt in range(0, 4):
                        attT_row(jt)
                    mt1_fill(1)
                    yT_ps = av_block(0)
                    stash_recip(0, yT_ps)
                    for jt in range(4, 8):
                        attT_row(jt)
                    for ib in range(IB):
                        s_mults(ib)
                        if ib + 1 < IB:
                            yT_ps = av_block(ib + 1)
                            stash_recip(ib + 1, yT_ps)
                        if ib == 0:
                            mt1_fill(1)
                        if ib + 2 < IB:
                            for jt in range(4 * (ib + 2), 4 * (ib + 2) + 4):
                                attT_row(jt)
                        outproj(ib)


_NC_CACHE = None


def get_nc() -> bass.Bass:
    global _NC_CACHE
    if _NC_CACHE is None:
        nc = bacc.Bacc()
        xT_d = nc.declare_dram_parameter("xT", [C, T], BF16, isOutput=False)
        wq_d = nc.declare_dram_parameter("wq", [C, MH], BF16, isOutput=False)
        wk_d = nc.declare_dram_parameter("wk", [C, MH], BF16, isOutput=False)
        wv_d = nc.declare_dram_parameter("wv", [C, MH], BF16, isOutput=False)
        wp_d = nc.declare_dram_parameter("wp", [MH, C], BF16, isOutput=False)
        bq_d = nc.declare_dram_parameter("bq", [MH], F32, isOutput=False)
        bk_d = nc.declare_dram_parameter("bk", [MH], F32, isOutput=False)
        bv_d = nc.declare_dram_parameter("bv", [MH], F32, isOutput=False)
        out_d = nc.declare_dram_parameter("out", [T, C], BF16, isOutput=True)
        mask_d = nc.inline_tensor(_causal_mask(), name="causal_mask")
        emit_kernel(
            nc, xT_d, wq_d, wk_d, wv_d, wp_d, bq_d, bk_d, bv_d, out_d, mask_d
        )
        nc.finalize()
        _NC_CACHE = nc
    return _NC_CACHE


def make_in_maps(x, Wq, bq, Wk, bk, Wv, bv, Wp, bp):
    in_maps = []
    for core in range(N_CORES):
        b, g = divmod(core, GROUPS)
        sl = slice(g * MH, (g + 1) * MH)
        in_maps.append(
            {
                "xT": np.ascontiguousarray(x[b].T).astype(NP_BF16),
                "wq": np.ascontiguousarray(Wq[:, sl]).astype(NP_BF16),
                "wk": np.ascontiguousarray(Wk[:, sl]).astype(NP_BF16),
                "wv": np.ascontiguousarray(Wv[:, sl]).astype(NP_BF16),
                "wp": np.ascontiguousarray(Wp[sl, :]).astype(NP_BF16),
                "bq": np.ascontiguousarray(bq[sl]).astype(np.float32),
                "bk": np.ascontiguousarray(bk[sl]).astype(np.float32),
                "bv": np.ascontiguousarray(bv[sl]).astype(np.float32),
            }
        )
    return in_maps


def kernel(x, Wq, bq, Wk, bk, Wv, bv, Wp, bp, _results_hook=None, _trace=False):
    x = np.asarray(x, dtype=np.float32)
    nc = get_nc()
    in_maps = make_in_maps(x, Wq, bq, Wk, bk, Wv, bv, Wp, bp)
    res = run_bass_kernel_spmd(
        nc, in_maps, core_ids=list(range(N_CORES)), trace=_trace
    )
    if _results_hook is not None:
        _results_hook(res)
    out = np.zeros((B, T, C), dtype=np.float32)
    for core in range(N_CORES):
        b = core // GROUPS
        out[b] += np.asarray(res.results[core]["out"], dtype=np.float32)
    out += np.asarray(bp, dtype=np.float32)[None, None, :]
    return out
